# revision 1
# baseline (speedup 1.0000x reference)
"""AdaAttN attention kernel for 8 TRN2 NeuronCores (v4).

Problem: nn_AdaAttN_29076928593982
  fc, fs, fcs: (4, 4096, 256) f32; Wf/Wg/Wh (256,256); bf/bg/bh (256,)
  Q = Wf@inorm(fc_t)+bf; K = Wg@inorm(fs_t)+bg; V = Wh@fs_t+bh
  A = softmax(Q K); M = A V; Var = A V^2 - M^2; S = sqrt(max(Var,1e-6))
  out = S * inorm(fcs_t) + M   (all in (b, t, d))

Sharding: data-parallel over (sample, query-half): core i -> sample i//2,
query rows [ (i%2)*2048, +2048 ). K/V replicated per sample. No collectives.

v4 over the baseline (302us):
  - Stats DMA lands DIRECTLY in the f32r projection operands through an
    f32-bitcast view; the sum pass is an in-place ACT Copy+accum that
    performs the f32r rounding cast, the sumsq pass is a DVE
    tensor_tensor_reduce. No chunk ring -> the DMA stream is never gated
    by engine-op completion (v3 trace: fc DMA pushes stalled to 62us
    behind an overloaded DVE stream).
  - V projection needs NO instance norm, so it runs chunk-by-chunk
    against the fs DMA stream; PE never idles long enough for the HAM
    clock-gate to re-throttle. V evacs on ACT, V^2 on GPSIMD: the DVE
    prologue stream stays light (it gates nothing).
  - fcsh is transposed HOST-side so the nfcs load is one contiguous DMA
    (the strided rearrange DMA was a 512-descriptor SBUF port storm that
    once stalled a DVE epilogue op 12.3us mid-attention).
  - All activations pinned to one ACT function table (Ln+Exp sqrt);
    the stock per-function table choice thrashed 1.28us table loads on
    the critical exp stream.
  - All attention matmuls f32r: the BIR verifier requires f32r operands
    to pair with f32r, and bf16 V costs ~1.7e-2 rel err through the
    Var = E[v^2] - M^2 cancellation (measured). f32r keeps ~3.7e-3.
"""
import sys

sys.path.insert(0, "/opt/trn_rl_repo")

import numpy as np

import concourse.bass as bass
import concourse.tile as tile
from concourse import bacc
from concourse import mybir
from concourse.bass_utils import run_bass_kernel_spmd

F32 = mybir.dt.float32
F32R = mybir.dt.float32r
AF = mybir.ActivationFunctionType
OP = mybir.AluOpType

P = 128          # partitions
D = 256          # feature dim
T = 4096         # tokens per sample
TH = 2048        # query tokens per core
CH = 2           # channel chunks (D // P)
NB = T // P      # tk chunks (32)
NQ = TH // 256   # tq chunks of 256 (8)
C0 = 110.0       # global softmax shift
EPS_IN = 1e-5
EPS_VAR = 1e-6
CK = 1024        # stats DMA chunk width
NCK = T // CK    # 4

TRACE = False    # test.py sets this to get exec_time_ns
TRACE_KW = {}

ACT_TABLE = "natural_log_exp_and_others"  # covers Copy/Exp/Identity/Ln/Square


class _Bacc(bacc.Bacc):
    """Bacc that pins all activations to one ACT function table.

    The stock pass assigns each activation the FIRST table containing its
    function, so Exp->exp_and_others but Ln->natural_log_exp_and_others:
    interleaved Ln/Exp then thrash 1.28us ACT_TABLE_LOADs on the critical
    exp stream (41 loads = 53us measured). Emptying every other table's
    function set (list positions kept, so set ids stay canonical) makes
    every function resolve to the one table, loaded once.
    """

    def insert_act_table_loads(self):
        import bass_rust as _bass_rust
        from concourse.hw_specs import get_activation_tables

        has_activation = any(
            isinstance(i, mybir.InstActivation)
            for b in self.main_func.blocks
            for i in b.instructions
        )
        if not has_activation:
            return
        tables = [
            (name, (fns if name == ACT_TABLE else set()))
            for name, fns in get_activation_tables(self.m.arch).items()
        ]
        _bass_rust.insert_act_table_loads(self, tables)


def _bcast_row(handle, offset, n):
    """AP reading a DRAM row of n elements broadcast across 128 partitions."""
    return bass.AP(tensor=handle, offset=offset, ap=[[0, P], [1, n]])


def build_nc():
    nc = _Bacc()

    fcT = nc.declare_dram_parameter("fcT", [D, T], F32, isOutput=False)
    fsT = nc.declare_dram_parameter("fsT", [D, T], F32, isOutput=False)
    fcsT = nc.declare_dram_parameter("fcsT", [D, T], F32, isOutput=False)
    # host-transposed to [p, n*d] so the load is one contiguous DMA
    fcsh = nc.declare_dram_parameter("fcsh", [P, (TH // P) * D], F32, isOutput=False)
    wfT = nc.declare_dram_parameter("wfT", [D, D], F32, isOutput=False)
    wgT = nc.declare_dram_parameter("wgT", [D, D], F32, isOutput=False)
    whT = nc.declare_dram_parameter("whT", [D, D], F32, isOutput=False)
    bq_e = nc.declare_dram_parameter("bq", [D, 1], F32, isOutput=False)
    bk_e = nc.declare_dram_parameter("bk", [D, 1], F32, isOutput=False)
    bv_e = nc.declare_dram_parameter("bv", [D], F32, isOutput=False)
    out_e = nc.declare_dram_parameter("out", [TH, D], F32, isOutput=True)

    scm = nc.dram_tensor("scm", [2, D], F32)  # fcs stats roundtrip scratch

    with tile.TileContext(nc) as tc:
        persist_cm = tc.tile_pool(name="persist", bufs=1)
        pp = persist_cm.__enter__()

        QTr = [pp.tile([P, TH], F32R, name=f"qtr{c}", tag=f"qtr{c}") for c in range(CH)]
        KTr = [pp.tile([P, T], F32R, name=f"ktr{c}", tag=f"ktr{c}") for c in range(CH)]
        Vr = pp.tile([P, NB, D + 2], F32R, name="vr", tag="vr")  # [V | ones | pad]
        V2r = pp.tile([P, NB, D], F32R, name="v2r", tag="v2r")
        bqe = [pp.tile([P, 1], F32, name=f"bqe{c}", tag=f"bqe{c}") for c in range(CH)]
        bke = [pp.tile([P, 1], F32, name=f"bke{c}", tag=f"bke{c}") for c in range(CH)]
        bv_bc = pp.tile([P, D], F32, name="bvbc", tag="bvbc")
        eps_t = pp.tile([P, 1], F32, name="epsin", tag="epsin")
        negc0_t = pp.tile([P, 1], F32, name="negc0", tag="negc0")

        # weight staging + folded copies, live for the whole prologue
        pw_cm = tc.tile_pool(name="pw", bufs=1)
        pw = pw_cm.__enter__()
        wf_sb = [pw.tile([P, D], F32, name=f"wf{c}", tag=f"wf{c}") for c in range(CH)]
        wg_sb = [pw.tile([P, D], F32, name=f"wg{c}", tag=f"wg{c}") for c in range(CH)]
        wh_sb = [pw.tile([P, D], F32, name=f"wh{c}", tag=f"wh{c}") for c in range(CH)]
        bq_sb = [pw.tile([P, 1], F32, name=f"bqs{c}", tag=f"bqs{c}") for c in range(CH)]
        bk_sb = [pw.tile([P, 1], F32, name=f"bks{c}", tag=f"bks{c}") for c in range(CH)]
        wq = [pw.tile([P, D], F32R, name=f"wq{c}", tag=f"wq{c}") for c in range(CH)]
        wk = [pw.tile([P, D], F32R, name=f"wk{c}", tag=f"wk{c}") for c in range(CH)]
        wv = [pw.tile([P, D], F32R, name=f"wv{c}", tag=f"wv{c}") for c in range(CH)]
        # 4-slot raw-chunk ring shared by the fs and fc streams, plus the
        # dead-output scratches for the sum (stats-only chunks) and square
        # passes; engine streams are prompt enough that the ring never
        # gates the DMA queue
        sqA = pw.tile([P, CK], F32, name="sqA", tag="sqA")
        sqB = pw.tile([P, CK], F32, name="sqB", tag="sqB")

        def ring_tile(nm):
            return pw.tile([P, CK], F32, name=nm, tag="ck", bufs=4)

        # weight DMAs on the gpsimd queue so the sync queue streams fs at t=0
        for c in range(CH):
            nc.gpsimd.dma_start(out=wh_sb[c], in_=whT[c * P : (c + 1) * P, :])
            nc.gpsimd.dma_start(out=wg_sb[c], in_=wgT[c * P : (c + 1) * P, :])
            nc.gpsimd.dma_start(out=wf_sb[c], in_=wfT[c * P : (c + 1) * P, :])
            nc.gpsimd.dma_start(out=bq_sb[c], in_=bq_e[c * P : (c + 1) * P, :])
            nc.gpsimd.dma_start(out=bk_sb[c], in_=bk_e[c * P : (c + 1) * P, :])
        nc.gpsimd.dma_start(out=bv_bc, in_=_bcast_row(bv_e, 0, D))

        nc.vector.memset(eps_t, EPS_IN)
        nc.vector.memset(negc0_t, -C0)
        ones_f32 = pw.tile([P, NB * 2], F32, name="ones32", tag="ones32")
        nc.vector.memset(ones_f32, 1.0)
        nc.vector.tensor_copy(
            Vr[:, :, D : D + 2], ones_f32.rearrange("p (n two) -> p n two", two=2)
        )
        for c in range(CH):
            nc.vector.tensor_copy(wv[c], wh_sb[c])

        def sum_round(dst_f32r, src_raw, acc_col):
            """ACT Copy raw -> f32r: performs the rounding cast (matmul-legal)
            and accumulates the per-channel sum in one op."""
            nc.scalar.activation(dst_f32r, src_raw, AF.Copy, accum_out=acc_col)

        def sum_only(src_raw, acc_col):
            """Sum pass for stats-only chunks; the Copy output is dead."""
            nc.scalar.activation(sqA, src_raw, AF.Copy, accum_out=acc_col)

        def sumsq(src, acc_col, out=None):
            nc.vector.scalar_tensor_tensor(
                out if out is not None else sqB,
                src, 0.0, src, op0=OP.add, op1=OP.mult, accum_out=acc_col,
            )

        def inv_std(ring, name, acc_s, acc_q):
            """mean + inv_std from chunk accumulators.
            inv_std = Exp(-0.5*Ln(var+eps)): Ln/Exp share the pinned table."""
            mean, invs = [], []
            for c in range(CH):
                m = ring.tile([P, 1], F32, name=f"{name}m{c}", tag=f"{name}m{c}")
                nc.vector.reduce_sum(m, acc_s[c], axis=mybir.AxisListType.X)
                nc.vector.tensor_scalar_mul(m, m, 1.0 / T)
                v = ring.tile([P, 1], F32, name=f"{name}v{c}", tag=f"{name}v{c}")
                nc.vector.reduce_sum(v, acc_q[c], axis=mybir.AxisListType.X)
                nc.vector.tensor_scalar_mul(v, v, 1.0 / T)
                msq = ring.tile([P, 1], F32, name=f"{name}msq{c}", tag=f"{name}msq{c}")
                nc.vector.tensor_mul(msq, m, m)
                nc.vector.tensor_sub(v, v, msq)
                nc.scalar.activation(v, v, AF.Ln, bias=eps_t)
                nc.scalar.activation(v, v, AF.Exp, scale=-0.5)
                mean.append(m)
                invs.append(v)
            return mean, invs

        # ---------------- fs phase: V proj streamed against the DMA -------
        pfs_cm = tc.tile_pool(name="pfs", bufs=1)
        pfs = pfs_cm.__enter__()
        fsr = [pfs.tile([P, T], F32R, name=f"fsr{c}", tag=f"fsr{c}") for c in range(CH)]
        acc_s_fs = [pfs.tile([P, NCK], F32, name=f"fsas{c}", tag=f"fsas{c}") for c in range(CH)]
        acc_q_fs = [pfs.tile([P, NCK], F32, name=f"fsaq{c}", tag=f"fsaq{c}") for c in range(CH)]

        psv_cm = tc.tile_pool(name="psv", bufs=2, space="PSUM")
        psv = psv_cm.__enter__()

        for k in range(NCK):
            ksl = slice(k * CK, (k + 1) * CK)
            fs_ck = []
            for c in range(CH):
                t = ring_tile(f"fsck{k}{c}")
                nc.sync.dma_start(out=t, in_=fsT[c * P : (c + 1) * P, ksl])
                fs_ck.append(t)
            for c in range(CH):
                sum_round(fsr[c][:, ksl], fs_ck[c], acc_s_fs[c][:, k : k + 1])
                sumsq(fs_ck[c], acc_q_fs[c][:, k : k + 1])
            # V proj for the 8 token blocks this chunk completes
            # (V = Wh @ fs + bh has no instance norm: no stats dependency)
            for tb in range(8 * k, 8 * (k + 1)):
                pv = psv.tile([P, D], F32, name=f"pv{tb}", tag="pv")
                sl = slice(tb * P, (tb + 1) * P)
                nc.tensor.matmul(pv, fsr[0][:, sl], wv[0], start=True, stop=False)
                nc.tensor.matmul(pv, fsr[1][:, sl], wv[1], start=False, stop=True)
                if tb % 2 == 0:
                    nc.scalar.activation(Vr[:, tb, 0:D], pv, AF.Copy)
                else:
                    nc.vector.tensor_copy(Vr[:, tb, 0:D], pv)
                nc.vector.tensor_mul(V2r[:, tb, :], Vr[:, tb, 0:D], Vr[:, tb, 0:D])

        # fc DMAs queue right behind fs so the stream never idles; the
        # engine ops are deferred into the K-proj interleave below.
        # cols 0:TH (near half, host-permuted) feed Q proj via fcr.
        pfc_cm = tc.tile_pool(name="pfc", bufs=1)
        pfc = pfc_cm.__enter__()
        fcr = [pfc.tile([P, TH], F32R, name=f"fcr{c}", tag=f"fcr{c}") for c in range(CH)]
        acc_s_fc = [pfc.tile([P, NCK], F32, name=f"fcas{c}", tag=f"fcas{c}") for c in range(CH)]
        acc_q_fc = [pfc.tile([P, NCK], F32, name=f"fcaq{c}", tag=f"fcaq{c}") for c in range(CH)]
        fc_ck = []
        for k in range(NCK):
            for c in range(CH):
                t = ring_tile(f"fcck{k}{c}")
                nc.sync.dma_start(
                    out=t, in_=fcT[c * P : (c + 1) * P, k * CK : (k + 1) * CK]
                )
                fc_ck.append(t)

        psb_cm = tc.tile_pool(name="psb", bufs=1, space="PSUM")
        psb = psb_cm.__enter__()

        # fs stats -> folded K weights + bias
        m_s, i_s = inv_std(pfs, "fs", acc_s_fs, acc_q_fs)
        for c in range(CH):
            nc.vector.tensor_scalar_mul(wk[c], wg_sb[c], i_s[c])
        m_sr = [pfs.tile([P, 2], F32R, name=f"fsmr{c}", tag=f"fsmr{c}") for c in range(CH)]
        for c in range(CH):
            nc.vector.tensor_copy(m_sr[c], m_s[c].to_broadcast((P, 2)))
        for oc in range(CH):
            pb = psb.tile([P, 2], F32, name=f"pbk{oc}", tag="pbk")
            nc.tensor.matmul(pb, wk[0][:, oc * P : (oc + 1) * P], m_sr[0], start=True, stop=False)
            nc.tensor.matmul(pb, wk[1][:, oc * P : (oc + 1) * P], m_sr[1], start=False, stop=True)
            nc.vector.tensor_sub(bke[oc], bk_sb[oc], pb[:, 0:1])

        psk_cm = tc.tile_pool(name="psk", bufs=3, space="PSUM")
        psk = psk_cm.__enter__()

        # K^T projection (o, tk) over full T, fc stats ops interleaved so
        # each engine stream roughly matches DMA arrival order
        kproj = [(oc, tch) for oc in range(CH) for tch in range(T // 512)]
        for i, (oc, tch) in enumerate(kproj):
            if i % 2 == 0 and i // 2 < len(fc_ck):
                k, c = divmod(i // 2, CH)
                raw = fc_ck[i // 2]
                if k * CK < TH:  # near half: rounded copy feeds Q proj
                    sum_round(
                        fcr[c][:, k * CK : (k + 1) * CK], raw,
                        acc_s_fc[c][:, k : k + 1],
                    )
                else:
                    sum_only(raw, acc_s_fc[c][:, k : k + 1])
                sumsq(raw, acc_q_fc[c][:, k : k + 1])
            pk = psk.tile([P, 512], F32, name=f"pk{oc}_{tch}", tag="pk")
            sl = slice(tch * 512, (tch + 1) * 512)
            nc.tensor.matmul(
                pk, wk[0][:, oc * P : (oc + 1) * P], fsr[0][:, sl],
                start=True, stop=False,
            )
            nc.tensor.matmul(
                pk, wk[1][:, oc * P : (oc + 1) * P], fsr[1][:, sl],
                start=False, stop=True,
            )
            if tch % 2 == 0:
                nc.scalar.activation(KTr[oc][:, sl], pk, AF.Identity, bias=bke[oc])
            else:
                nc.vector.tensor_scalar_add(KTr[oc][:, sl], pk, bke[oc])

        # fc stats -> folded Q weights + bias
        m_c, i_c = inv_std(pfc, "fc", acc_s_fc, acc_q_fc)
        for c in range(CH):
            nc.vector.tensor_scalar_mul(wq[c], wf_sb[c], i_c[c])
        m_r = [pfc.tile([P, 2], F32R, name=f"fcmr{c}", tag=f"fcmr{c}") for c in range(CH)]
        for c in range(CH):
            nc.vector.tensor_copy(m_r[c], m_c[c].to_broadcast((P, 2)))
        for oc in range(CH):
            pb = psb.tile([P, 2], F32, name=f"pbq{oc}", tag="pbq")
            nc.tensor.matmul(pb, wq[0][:, oc * P : (oc + 1) * P], m_r[0], start=True, stop=False)
            nc.tensor.matmul(pb, wq[1][:, oc * P : (oc + 1) * P], m_r[1], start=False, stop=True)
            nc.vector.tensor_sub(bqe[oc], bq_sb[oc], pb[:, 0:1])

        # Q^T projection: core's own half is host-permuted to cols 0:TH
        for oc in range(CH):
            for tch in range(TH // 512):
                pq = psk.tile([P, 512], F32, name=f"pq{oc}_{tch}", tag="pk")
                sl = slice(tch * 512, (tch + 1) * 512)
                nc.tensor.matmul(
                    pq, wq[0][:, oc * P : (oc + 1) * P], fcr[0][:, sl],
                    start=True, stop=False,
                )
                nc.tensor.matmul(
                    pq, wq[1][:, oc * P : (oc + 1) * P], fcr[1][:, sl],
                    start=False, stop=True,
                )
                if tch % 2 == 0:
                    nc.scalar.activation(QTr[oc][:, sl], pq, AF.Identity, bias=bqe[oc])
                else:
                    nc.vector.tensor_scalar_add(QTr[oc][:, sl], pq, bqe[oc])

        # prologue scratch + PSUM no longer needed (LIFO order)
        psk_cm.__exit__(None, None, None)
        psb_cm.__exit__(None, None, None)
        psv_cm.__exit__(None, None, None)
        pfc_cm.__exit__(None, None, None)
        pfs_cm.__exit__(None, None, None)
        pw_cm.__exit__(None, None, None)

        # ---------------- attention (fcs stats folded in) ------------------
        with tc.tile_pool(name="pfcs", bufs=1) as pfcs, tc.tile_pool(
            name="sts", bufs=5
        ) as sts, tc.tile_pool(name="epi", bufs=3) as epi, tc.tile_pool(
            name="psl", bufs=3, space="PSUM"
        ) as psl, tc.tile_pool(name="pmv", bufs=1, space="PSUM") as pmv, tc.tile_pool(
            name="pv2", bufs=1, space="PSUM"
        ) as pv2:
            nfcs = pfcs.tile([P, TH // P, D], F32, name="nfcs", tag="nfcs")
            m_bc = pfcs.tile([P, D], F32, name="mbc", tag="mbc")
            i_bc = pfcs.tile([P, D], F32, name="ibc", tag="ibc")
            csqA = pfcs.tile([P, CK], F32, name="csqA", tag="csqA")
            csqB = pfcs.tile([P, CK], F32, name="csqB", tag="csqB")
            acc_s_cs = [pfcs.tile([P, NCK], F32, name=f"csas{c}", tag=f"csas{c}") for c in range(CH)]
            acc_q_cs = [pfcs.tile([P, NCK], F32, name=f"csaq{c}", tag=f"csaq{c}") for c in range(CH)]
            # all fcs chunk DMAs + the contiguous nfcs load queue up front;
            # the 6-deep ring means only the last two chunks wait on the
            # first chunk ops (which run early in the q0 loop)
            cs_ck = []
            for k in range(NCK):
                for c in range(CH):
                    t = pfcs.tile([P, CK], F32, name=f"csck{k}{c}", tag="csck", bufs=6)
                    nc.sync.dma_start(
                        out=t, in_=fcsT[c * P : (c + 1) * P, k * CK : (k + 1) * CK]
                    )
                    cs_ck.append(t)
            nc.sync.dma_start(
                out=nfcs, in_=fcsh[:, :].rearrange("p (n d) -> p n d", d=D)
            )

            def emit_fcs_op(i):
                k, c = divmod(i, CH)
                nc.scalar.activation(
                    csqA, cs_ck[i], AF.Copy, accum_out=acc_s_cs[c][:, k : k + 1]
                )
                sumsq(cs_ck[i], acc_q_cs[c][:, k : k + 1], out=csqB)

            def emit_fcs_reduce():
                m_cs, i_cs = inv_std(pfcs, "cs", acc_s_cs, acc_q_cs)
                for c in range(CH):
                    nc.gpsimd.dma_start(out=scm[0, c * P : (c + 1) * P], in_=m_cs[c])
                    nc.gpsimd.dma_start(out=scm[1, c * P : (c + 1) * P], in_=i_cs[c])
                nc.gpsimd.dma_start(out=m_bc, in_=_bcast_row(scm, 0, D))
                nc.gpsimd.dma_start(out=i_bc, in_=_bcast_row(scm, D, D))

            def emit_nfcs_norm(b):
                nc.gpsimd.tensor_sub(nfcs[:, b, :], nfcs[:, b, :], m_bc)
                nc.gpsimd.tensor_mul(nfcs[:, b, :], nfcs[:, b, :], i_bc)

            for q in range(NQ):  # tq chunks of 256
                qsl = slice(q * 256, (q + 1) * 256)
                mv = [pmv.tile([P, D + 2], F32, name=f"mv{q}_{i}", tag=f"mv{i}") for i in range(2)]
                v2 = [pv2.tile([P, D], F32, name=f"v2_{q}_{i}", tag=f"v2{i}") for i in range(2)]
                sts_tiles = [None] * (NB // 2)

                def emit_logits(jp, q=q, qsl=qsl, sts_tiles=sts_tiles):
                    pl = psl.tile([P, 512], F32, name=f"pl{q}_{jp}", tag="pl")
                    for h in range(2):
                        j = 2 * jp + h
                        osl = slice(h * 256, (h + 1) * 256)
                        nc.tensor.matmul(
                            pl[:, osl], KTr[0][:, j * P : (j + 1) * P], QTr[0][:, qsl],
                            start=True, stop=False,
                        )
                        nc.tensor.matmul(
                            pl[:, osl], KTr[1][:, j * P : (j + 1) * P], QTr[1][:, qsl],
                            start=False, stop=True,
                        )
                    st = sts.tile([P, 512], F32R, name="st", tag="st")
                    nc.scalar.activation(st, pl, AF.Exp, bias=negc0_t)
                    sts_tiles[jp] = st

                def emit_av(jp, q=q, mv=mv, v2=v2, sts_tiles=sts_tiles):
                    st = sts_tiles[jp]
                    for h in range(2):
                        j = 2 * jp + h
                        for b in range(2):
                            lhs = st[:, h * 256 + b * P : h * 256 + (b + 1) * P]
                            nc.tensor.matmul(
                                mv[b], lhs, Vr[:, j, :],
                                start=(j == 0), stop=(j == NB - 1),
                            )
                            nc.tensor.matmul(
                                v2[b], lhs, V2r[:, j, :],
                                start=(j == 0), stop=(j == NB - 1),
                            )

                emit_logits(0)
                for jp in range(1, NB // 2):
                    emit_logits(jp)
                    emit_av(jp - 1)
                    # fcs stats ops stream through q-chunk 0 paced to DMA
                    # arrival; reduce + first norms land before the epilogue
                    if q == 0:
                        if 5 <= jp <= 12:
                            emit_fcs_op(jp - 5)
                        elif jp == 13:
                            emit_fcs_reduce()
                emit_av(NB // 2 - 1)
                if q == 0:
                    for b in range(4):
                        emit_nfcs_norm(b)
                elif q <= 6:
                    emit_nfcs_norm(2 * q + 2)
                    emit_nfcs_norm(2 * q + 3)

                for b in range(2):
                    qb = q * 2 + b
                    # evacuate PSUM right away so the next chunk's matmuls
                    # reuse the banks without waiting on the epilogue
                    mve = epi.tile([P, D + 2], F32, name="mve", tag="mve")
                    nc.vector.tensor_copy(mve, mv[b])
                    v2e = epi.tile([P, D], F32, name="v2e", tag="v2e")
                    nc.vector.tensor_copy(v2e, v2[b])
                    recip = epi.tile([P, 1], F32, name="recip", tag="recip")
                    nc.vector.reciprocal(recip, mve[:, D : D + 1])
                    Mt = epi.tile([P, D], F32, name="Mt", tag="Mt")
                    nc.vector.tensor_scalar_mul(Mt, mve[:, 0:D], recip)  # unbiased M
                    Msq = epi.tile([P, D], F32, name="Msq", tag="Msq")
                    nc.vector.tensor_mul(Msq, Mt, Mt)
                    # Var -> v2e (in place), clamp, S = Exp(0.5*Ln(Var))
                    nc.vector.scalar_tensor_tensor(
                        v2e, v2e, recip, Msq, op0=OP.mult, op1=OP.subtract
                    )
                    nc.vector.tensor_scalar_max(v2e, v2e, EPS_VAR)
                    nc.scalar.activation(Msq, v2e, AF.Ln)
                    nc.scalar.activation(Msq, Msq, AF.Exp, scale=0.5)
                    # out = S*nfcs + M + bh (final two on GPSIMD: SBUF-only)
                    Mb = epi.tile([P, D], F32, name="Mb", tag="Mb")
                    nc.vector.tensor_add(Mb, Mt, bv_bc)
                    nc.gpsimd.tensor_mul(Msq, Msq, nfcs[:, qb, :])
                    nc.gpsimd.tensor_add(Msq, Msq, Mb)
                    nc.sync.dma_start(out=out_e[qb * P : (qb + 1) * P, :], in_=Msq)

        persist_cm.__exit__(None, None, None)

    nc.compile()
    return nc


_CACHE = {}


def _get_nc():
    if "nc" not in _CACHE:
        _CACHE["nc"] = build_nc()
    return _CACHE["nc"]


def kernel(**inputs):
    fc = np.ascontiguousarray(np.asarray(inputs["fc"], dtype=np.float32))
    fs = np.ascontiguousarray(np.asarray(inputs["fs"], dtype=np.float32))
    fcs = np.ascontiguousarray(np.asarray(inputs["fcs"], dtype=np.float32))
    Wf = np.asarray(inputs["Wf"], dtype=np.float32)
    bf = np.asarray(inputs["bf"], dtype=np.float32)
    Wg = np.asarray(inputs["Wg"], dtype=np.float32)
    bg = np.asarray(inputs["bg"], dtype=np.float32)
    Wh = np.asarray(inputs["Wh"], dtype=np.float32)
    bh = np.asarray(inputs["bh"], dtype=np.float32)

    wfT = np.ascontiguousarray(Wf.T)
    wgT = np.ascontiguousarray(Wg.T)
    whT = np.ascontiguousarray(Wh.T)
    bq = np.ascontiguousarray(bf.reshape(D, 1))
    bk = np.ascontiguousarray(bg.reshape(D, 1))

    in_maps = []
    for core in range(8):
        s, h = divmod(core, 2)
        fcT_s = fc[s].T  # (D, T)
        if h == 0:
            fcT_perm = np.ascontiguousarray(fcT_s)
        else:
            fcT_perm = np.ascontiguousarray(
                np.concatenate([fcT_s[:, TH:], fcT_s[:, :TH]], axis=1)
            )
        # [TH, D] -> [P, (TH//P)*D]: token block on the middle axis so the
        # device-side load is contiguous per partition
        fcsh_t = np.ascontiguousarray(
            fcs[s, h * TH : (h + 1) * TH, :]
            .reshape(TH // P, P, D)
            .transpose(1, 0, 2)
            .reshape(P, (TH // P) * D)
        )
        in_maps.append(
            {
                "fcT": fcT_perm,
                "fsT": np.ascontiguousarray(fs[s].T),
                "fcsT": np.ascontiguousarray(fcs[s].T),
                "fcsh": fcsh_t,
                "wfT": wfT,
                "wgT": wgT,
                "whT": whT,
                "bq": bq,
                "bk": bk,
                "bv": bh,
            }
        )

    nc = _get_nc()
    res = run_bass_kernel_spmd(
        nc, in_maps, core_ids=list(range(8)), trace=TRACE, **TRACE_KW
    )
    if TRACE:
        _CACHE["last_result"] = res

    out = np.empty((4, T, D), np.float32)
    for core in range(8):
        s, h = divmod(core, 2)
        out[s, h * TH : (h + 1) * TH, :] = res.results[core]["out"]
    return out



# revision 31
# speedup vs baseline: 1.0922x; 1.0922x over previous
"""AdaAttN attention kernel for 8 TRN2 NeuronCores (v5).

Problem: nn_AdaAttN_29076928593982
  fc, fs, fcs: (4, 4096, 256) f32; Wf/Wg/Wh (256,256); bf/bg/bh (256,)
  Q = Wf@inorm(fc_t)+bf; K = Wg@inorm(fs_t)+bg; V = Wh@fs_t+bh
  A = softmax(Q K); M = A V; Var = A V^2 - M^2; S = sqrt(max(Var,1e-6))
  out = S * inorm(fcs_t) + M   (all in (b, t, d))

Sharding: data-parallel over (sample, query-half): core i -> sample i//2,
query rows [ (i%2)*2048, +2048 ). K/V replicated per sample. No collectives.

v5 over v4 (287.7us):
  - Softmax shift-invariance per query row: the K-side bias bg AND the
    K-side mean subtraction only contribute per-query additive constants
    to the logits, which cancel in softmax. So K is never projected:
    logits = Qs^T @ fs_raw with Qs = diag(i_s) * (W2^T @ fc_raw + b2),
    W2 = (Wg^T Wf diag(i_c))^T a 256x256 weight-space product (1k PE
    rows) and b2 = Wg^T bqe. Kills the 16k-row K projection, the KTr
    tile, and v4's 13.3us wait-for-stats PE idle (HAM re-throttle).
  - fc streams FIRST, fs second; the W2/V-projection matmuls fill the
    fs DMA window so the PE enters attention warm (v4 ran its first
    57us at the cold 1.2GHz HAM clock).
  - The fs-side scale i_s is applied to the 2048-query Qs (not the
    4096-key side), fused into the PSUM evacuation (scale+bias), and
    only the first 512-query slice gates attention start.
  - GPSIMD does NO bulk compute: its TT bursts grab the DVE shared
    SBUF port pair and fully block DVE copy/tensor_scalar ops (v4 lost
    10us to one such 14-op burst). Everything balances across ACT/DVE,
    with paired two-block V evac/square ops to cut op count.
  - Epilogue folds the fcs instance norm in algebraically:
    out = Sa*fcs_raw + (M + bh - Sa*m_cs), Sa = S*i_cs — no separate
    normalization pass; the PSUM-freeing copies are emitted first so
    the PE never waits on the rest of the chain.
  - Logits run 3 tasks ahead of the AV stream so the PE never drains
    at chunk boundaries; V(k3)/fcs-stats work interleaves into chunk 0
    at emission positions chosen to meet their just-in-time deadlines.
"""
import sys

sys.path.insert(0, "/opt/trn_rl_repo")

import numpy as np

import concourse.bass as bass
import concourse.tile as tile
from concourse import bacc
from concourse import mybir
from concourse.bass_utils import run_bass_kernel_spmd

F32 = mybir.dt.float32
F32R = mybir.dt.float32r
AF = mybir.ActivationFunctionType
OP = mybir.AluOpType

P = 128          # partitions
D = 256          # feature dim
T = 4096         # tokens per sample
TH = 2048        # query tokens per core
CH = 2           # channel chunks (D // P)
NB = T // P      # key blocks (32)
NQ = TH // 256   # q chunks of 256 (8)
NJP = NB // 2    # jp tasks per chunk (16)
CK = 1024        # stream DMA chunk width
NCK = T // CK    # 4
C0 = 110.0       # global softmax shift
EPS_IN = 1e-5
EPS_VAR = 1e-6
LA = 3           # logits lookahead (tasks) over the AV stream

TRACE = False    # test.py sets this to get exec_time_ns
TRACE_KW = {}

ACT_TABLE = "natural_log_exp_and_others"  # covers Copy/Exp/Identity/Ln/Square


class _Bacc(bacc.Bacc):
    """Bacc that pins all activations to one ACT function table.

    The stock pass assigns each activation the FIRST table containing its
    function, so Exp->exp_and_others but Ln->natural_log_exp_and_others:
    interleaved Ln/Exp then thrash 1.28us ACT_TABLE_LOADs on the critical
    exp stream. Emptying every other table's function set (list positions
    kept, so set ids stay canonical) makes every function resolve to the
    one table, loaded once.
    """

    def insert_act_table_loads(self):
        import bass_rust as _bass_rust
        from concourse.hw_specs import get_activation_tables

        has_activation = any(
            isinstance(i, mybir.InstActivation)
            for b in self.main_func.blocks
            for i in b.instructions
        )
        if not has_activation:
            return
        tables = [
            (name, (fns if name == ACT_TABLE else set()))
            for name, fns in get_activation_tables(self.m.arch).items()
        ]
        _bass_rust.insert_act_table_loads(self, tables)


def _bcast_row(handle, offset, n):
    """AP reading a DRAM row of n elements broadcast across 128 partitions."""
    return bass.AP(tensor=handle, offset=offset, ap=[[0, P], [1, n]])


def build_nc():
    nc = _Bacc()

    fcT = nc.declare_dram_parameter("fcT", [D, T], F32, isOutput=False)
    fsT = nc.declare_dram_parameter("fsT", [D, T], F32, isOutput=False)
    fcsT = nc.declare_dram_parameter("fcsT", [D, T], F32, isOutput=False)
    # host-transposed to [p, n*d] so the load is one contiguous DMA
    fcsh = nc.declare_dram_parameter("fcsh", [P, (TH // P) * D], F32, isOutput=False)
    wfO = nc.declare_dram_parameter("wfO", [D, D], F32, isOutput=False)   # Wf [o,c]
    wfT = nc.declare_dram_parameter("wfT", [D, D], F32, isOutput=False)   # Wf^T [c,o]
    wgO = nc.declare_dram_parameter("wgO", [D, D], F32, isOutput=False)   # Wg [o,c2]
    whT = nc.declare_dram_parameter("whT", [D, D], F32, isOutput=False)   # Wh^T [c2,d]
    bq_e = nc.declare_dram_parameter("bq", [D, 1], F32, isOutput=False)   # bf
    bv_e = nc.declare_dram_parameter("bv", [D], F32, isOutput=False)      # bh
    out_e = nc.declare_dram_parameter("out", [TH, D], F32, isOutput=True)

    scm = nc.dram_tensor("scm", [2, D], F32)  # fcs stats broadcast roundtrip

    with tile.TileContext(nc) as tc:
        persist_cm = tc.tile_pool(name="persist", bufs=1)
        pp = persist_cm.__enter__()

        # fs, rounded to f32r by the stats ACT Copy; doubles as the K
        # matrix (lhsT of logits). The BIR verifier requires every location
        # an f32r matmul consumes to be written ONLY by rounding ops, so
        # the DMA lands in a ring and the rounding Copy (which also
        # accumulates the per-channel sum) produces fsr.
        fsr = [pp.tile([P, T], F32R, name=f"fsr{c}", tag=f"fsr{c}") for c in range(CH)]
        Vr = pp.tile([P, NB, D + 2], F32R, name="vr", tag="vr")  # [V | ones | pad]
        V2r = pp.tile([P, NB, D], F32R, name="v2r", tag="v2r")
        Qs = [pp.tile([P, TH], F32R, name=f"qs{c}", tag=f"qs{c}") for c in range(CH)]
        wv = [pp.tile([P, D], F32R, name=f"wv{c}", tag=f"wv{c}") for c in range(CH)]
        bv_bc = pp.tile([P, D], F32, name="bvbc", tag="bvbc")
        eps_t = pp.tile([P, 1], F32, name="epsin", tag="epsin")
        negc0_t = pp.tile([P, 1], F32, name="negc0", tag="negc0")

        # ---------------- prologue pool: fc stream, Q-path, fs stream ----
        pfc_cm = tc.tile_pool(name="pfc", bufs=1)
        pf = pfc_cm.__enter__()
        # only the core's own query half feeds matmuls; the far half is
        # stats-only and never materializes rounded
        fcr = [pf.tile([P, TH], F32R, name=f"fcr{c}", tag=f"fcr{c}") for c in range(CH)]
        wfO_sb = [pf.tile([P, D], F32, name=f"wfo{o}", tag=f"wfo{o}") for o in range(CH)]
        wfT_sb = [pf.tile([P, D], F32, name=f"wft{c}", tag=f"wft{c}") for c in range(CH)]
        wgO_sb = [pf.tile([P, D], F32, name=f"wgo{o}", tag=f"wgo{o}") for o in range(CH)]
        whT_sb = [pf.tile([P, D], F32, name=f"wht{c}", tag=f"wht{c}") for c in range(CH)]
        wfO_r = [pf.tile([P, D], F32R, name=f"wfor{o}", tag=f"wfor{o}") for o in range(CH)]
        wfT_r = [pf.tile([P, D], F32R, name=f"wftr{c}", tag=f"wftr{c}") for c in range(CH)]
        wgO_r = [pf.tile([P, D], F32R, name=f"wgor{o}", tag=f"wgor{o}") for o in range(CH)]
        bq_sb = [pf.tile([P, 1], F32, name=f"bqs{o}", tag=f"bqs{o}") for o in range(CH)]
        W2T = [pf.tile([P, D], F32R, name=f"w2t{c}", tag=f"w2t{c}") for c in range(CH)]
        acc_s_fc = [pf.tile([P, NCK], F32, name=f"asfc{c}", tag=f"asfc{c}") for c in range(CH)]
        acc_q_fc = [pf.tile([P, NCK], F32, name=f"aqfc{c}", tag=f"aqfc{c}") for c in range(CH)]
        acc_s_fs = [pf.tile([P, NCK + 1], F32, name=f"asfs{c}", tag=f"asfs{c}") for c in range(CH)]
        acc_q_fs = [pf.tile([P, NCK + 1], F32, name=f"aqfs{c}", tag=f"aqfs{c}") for c in range(CH)]
        sqD = pf.tile([P, CK], F32, name="sqD", tag="sqD")  # DVE dead out
        sqA = pf.tile([P, CK], F32, name="sqA", tag="sqA")  # ACT dead out
        iA = [pf.tile([P, 1], F32, name=f"ia{c}", tag=f"ia{c}") for c in range(CH)]
        iB = [pf.tile([P, 1], F32, name=f"ib{c}", tag=f"ib{c}") for c in range(CH)]
        imA = [pf.tile([P, 1], F32, name=f"ima{c}", tag=f"ima{c}") for c in range(CH)]
        mA = [pf.tile([P, 1], F32, name=f"ma{c}", tag=f"ma{c}") for c in range(CH)]
        vA = [pf.tile([P, 1], F32, name=f"va{c}", tag=f"va{c}") for c in range(CH)]
        imr = [pf.tile([P, 2], F32R, name=f"imr{c}", tag=f"imr{c}") for c in range(CH)]
        bqe = [pf.tile([P, 1], F32, name=f"bqe{o}", tag=f"bqe{o}") for o in range(CH)]
        bqe_r = [pf.tile([P, 2], F32R, name=f"bqer{o}", tag=f"bqer{o}") for o in range(CH)]
        b2t = [pf.tile([P, 1], F32, name=f"b2t{c}", tag=f"b2t{c}") for c in range(CH)]
        b2s = [pf.tile([P, 1], F32, name=f"b2s{c}", tag=f"b2s{c}") for c in range(CH)]
        ones_f32 = pf.tile([P, NB * 2], F32, name="ones32", tag="ones32")

        def ring_tile(nm):
            return pf.tile([P, CK], F32, name=nm, tag="ck", bufs=4)

        # PSUM pools: pvp outlives the others (V proj spans the fs stream)
        pvp_cm = tc.tile_pool(name="pvp", bufs=2, space="PSUM")
        pvp = pvp_cm.__enter__()
        pxt_cm = tc.tile_pool(name="pxt", bufs=1, space="PSUM")
        pxt = pxt_cm.__enter__()

        # weight + bias DMAs on the pool queue so sync streams fc at t=0
        for o in range(CH):
            sl = slice(o * P, (o + 1) * P)
            nc.gpsimd.dma_start(out=wfO_sb[o], in_=wfO[sl, :])
            nc.gpsimd.dma_start(out=wgO_sb[o], in_=wgO[sl, :])
            nc.gpsimd.dma_start(out=wfT_sb[o], in_=wfT[sl, :])
            nc.gpsimd.dma_start(out=whT_sb[o], in_=whT[sl, :])
            nc.gpsimd.dma_start(out=bq_sb[o], in_=bq_e[sl, :])
        nc.gpsimd.dma_start(out=bv_bc, in_=_bcast_row(bv_e, 0, D))

        # input streams on the sync queue: fc first, then fs
        fc_ck, fs_ck = [], []
        for k in range(NCK):
            for c in range(CH):
                t = ring_tile(f"fcck{k}{c}")
                nc.sync.dma_start(
                    out=t, in_=fcT[c * P : (c + 1) * P, k * CK : (k + 1) * CK]
                )
                fc_ck.append(t)
        for k in range(NCK):
            for c in range(CH):
                t = ring_tile(f"fsck{k}{c}")
                nc.sync.dma_start(
                    out=t, in_=fsT[c * P : (c + 1) * P, k * CK : (k + 1) * CK]
                )
                fs_ck.append(t)

        nc.vector.memset(eps_t, EPS_IN)
        nc.vector.memset(negc0_t, -C0)
        nc.vector.memset(ones_f32, 1.0)
        nc.vector.tensor_copy(
            Vr[:, :, D : D + 2], ones_f32.rearrange("p (n two) -> p n two", two=2)
        )
        # rounded f32r weight copies
        for o in range(CH):
            nc.vector.tensor_copy(wfO_r[o], wfO_sb[o])
            nc.vector.tensor_copy(wgO_r[o], wgO_sb[o])
            nc.vector.tensor_copy(wfT_r[o], wfT_sb[o])
            nc.vector.tensor_copy(wv[o], whT_sb[o])

        # XT[c, c2] = sum_o Wf[o,c] Wg[o,c2]  (no stats dependency)
        xt_ps = []
        for c in range(CH):
            px = pxt.tile([P, D], F32, name=f"xt{c}", tag=f"xt{c}")
            csl = slice(c * P, (c + 1) * P)
            nc.tensor.matmul(px, wfO_r[0][:, csl], wgO_r[0], start=True, stop=False)
            nc.tensor.matmul(px, wfO_r[1][:, csl], wgO_r[1], start=False, stop=True)
            xt_ps.append(px)

        def round_sum(dst, raw, acc_col):
            """ACT Copy raw -> f32r: performs the rounding cast (matmul-legal)
            and accumulates the per-channel sum in one op."""
            nc.scalar.activation(dst, raw, AF.Copy, accum_out=acc_col)

        def sumsq(raw, acc_col):
            nc.vector.scalar_tensor_tensor(
                sqD, raw, 0.0, raw, op0=OP.add, op1=OP.mult, accum_out=acc_col
            )

        # fc stats, chunk-paced: ACT rounds+sums (own half) or sums (far
        # half, dead output); DVE sums squares
        for k in range(NCK):
            for c in range(CH):
                raw = fc_ck[k * CH + c]
                if k * CK < TH:
                    round_sum(
                        fcr[c][:, k * CK : (k + 1) * CK], raw,
                        acc_s_fc[c][:, k : k + 1],
                    )
                else:
                    round_sum(sqA, raw, acc_s_fc[c][:, k : k + 1])
                sumsq(raw, acc_q_fc[c][:, k : k + 1])

        # fc stats -> i_c, folded weight W2T, bias path
        for c in range(CH):
            nc.vector.reduce_sum(mA[c], acc_s_fc[c], axis=mybir.AxisListType.X)
            nc.vector.tensor_scalar_mul(mA[c], mA[c], 1.0 / T)
            nc.vector.reduce_sum(vA[c], acc_q_fc[c], axis=mybir.AxisListType.X)
            nc.vector.tensor_scalar_mul(vA[c], vA[c], 1.0 / T)
            nc.vector.tensor_mul(imA[c], mA[c], mA[c])
            nc.vector.tensor_sub(vA[c], vA[c], imA[c])
            nc.scalar.activation(vA[c], vA[c], AF.Ln, bias=eps_t)
            nc.scalar.activation(iA[c], vA[c], AF.Exp, scale=-0.5)
            nc.vector.tensor_mul(imA[c], mA[c], iA[c])
            nc.vector.tensor_copy(imr[c], imA[c].to_broadcast((P, 2)))
        # W2T = XT * i_c (per-partition scale), rounded to f32r
        for c in range(CH):
            nc.scalar.activation(W2T[c], xt_ps[c], AF.Identity, scale=iA[c])
        pxt_cm.__exit__(None, None, None)

        psm_cm = tc.tile_pool(name="psm", bufs=2, space="PSUM")
        psm = psm_cm.__enter__()
        # bqe[o] = bf[o] - sum_c Wf[o,c] i_c m_c
        for oc in range(CH):
            osl = slice(oc * P, (oc + 1) * P)
            pb = psm.tile([P, 2], F32, name=f"pbq{oc}", tag="pb")
            nc.tensor.matmul(pb, wfT_r[0][:, osl], imr[0], start=True, stop=False)
            nc.tensor.matmul(pb, wfT_r[1][:, osl], imr[1], start=False, stop=True)
            nc.vector.tensor_sub(bqe[oc], bq_sb[oc], pb[:, 0:1])
            nc.vector.tensor_copy(bqe_r[oc], bqe[oc].to_broadcast((P, 2)))
        # b2[c2] = sum_o Wg[o,c2] bqe[o]
        for c2 in range(CH):
            csl = slice(c2 * P, (c2 + 1) * P)
            pb = psm.tile([P, 2], F32, name=f"pb2{c2}", tag="pb")
            nc.tensor.matmul(pb, wgO_r[0][:, csl], bqe_r[0], start=True, stop=False)
            nc.tensor.matmul(pb, wgO_r[1][:, csl], bqe_r[1], start=False, stop=True)
            nc.vector.tensor_copy(b2t[c2], pb[:, 0:1])
        psm_cm.__exit__(None, None, None)

        # V projection (chunks 0..2) + fs stats, paced against the fs DMA.
        # ACT: round+sum; DVE: sumsq; V evac / V^2 in two-block pair ops
        # alternating ACT/DVE. The last fs chunk's stats are sub-chunked at
        # 512 so the attention-start chain fires right behind the stream;
        # its V work is deferred into the early attention task stream.
        def v_pair(pr, pool=None, tag="pv"):
            tb = 2 * pr
            pv = (pool or pvp).tile([P, 2, D], F32, name=f"pv{pr}", tag=tag)
            for h in range(2):
                sl = slice((tb + h) * P, (tb + h + 1) * P)
                nc.tensor.matmul(
                    pv[:, h], fsr[0][:, sl], wv[0], start=True, stop=False
                )
                nc.tensor.matmul(
                    pv[:, h], fsr[1][:, sl], wv[1], start=False, stop=True
                )
            return pv

        def v_pair_evac(pr, pv):
            tb = 2 * pr
            if pr % 2 == 0:
                nc.scalar.activation(Vr[:, tb : tb + 2, 0:D], pv, AF.Copy)
                nc.vector.tensor_mul(
                    V2r[:, tb : tb + 2, :], Vr[:, tb : tb + 2, 0:D],
                    Vr[:, tb : tb + 2, 0:D],
                )
            else:
                nc.vector.tensor_copy(Vr[:, tb : tb + 2, 0:D], pv)
                nc.scalar.activation(
                    V2r[:, tb : tb + 2, :], Vr[:, tb : tb + 2, 0:D], AF.Square
                )

        for k in range(NCK - 1):
            for c in range(CH):
                raw = fs_ck[k * CH + c]
                round_sum(
                    fsr[c][:, k * CK : (k + 1) * CK], raw,
                    acc_s_fs[c][:, k : k + 1],
                )
                sumsq(raw, acc_q_fs[c][:, k : k + 1])
            for pr in range(4 * k, 4 * (k + 1)):
                pv = v_pair(pr)
                v_pair_evac(pr, pv)
        # last fs chunk: stats sub-chunked at 512 for a short tail
        k = NCK - 1
        for c in range(CH):
            raw = fs_ck[k * CH + c]
            for a in range(2):
                lo = k * CK + a * 512
                round_sum(
                    fsr[c][:, lo : lo + 512], raw[:, a * 512 : (a + 1) * 512],
                    acc_s_fs[c][:, k + a : k + a + 1],
                )
                nc.vector.scalar_tensor_tensor(
                    sqD[:, 0:512], raw[:, a * 512 : (a + 1) * 512], 0.0,
                    raw[:, a * 512 : (a + 1) * 512],
                    op0=OP.add, op1=OP.mult,
                    accum_out=acc_q_fs[c][:, k + a : k + a + 1],
                )

        # fs stats -> i_s (the only thing attention start waits on)
        for c in range(CH):
            m = iB[c]  # mean lands in the i slot first (overwritten below)
            nc.vector.reduce_sum(m, acc_s_fs[c], axis=mybir.AxisListType.X)
            nc.vector.tensor_scalar_mul(m, m, 1.0 / T)
            v = vA[c]
            nc.vector.reduce_sum(v, acc_q_fs[c], axis=mybir.AxisListType.X)
            nc.vector.tensor_scalar_mul(v, v, 1.0 / T)
            nc.vector.tensor_mul(m, m, m)
            nc.vector.tensor_sub(v, v, m)
            nc.scalar.activation(v, v, AF.Ln, bias=eps_t)
            nc.scalar.activation(iB[c], v, AF.Exp, scale=-0.5)
            nc.vector.tensor_mul(b2s[c], b2t[c], iB[c])

        # Qs = (W2T^T @ fc_raw) * i_s + b2*i_s, evacuated slice-by-slice
        # with the scale+bias fused; slice 0 (both c2) first — it alone
        # gates the first logits.
        pqp_cm = tc.tile_pool(name="pqp", bufs=3, space="PSUM")
        pqp = pqp_cm.__enter__()
        for qs in range(TH // 512):
            for c2 in range(CH):
                csl = slice(c2 * P, (c2 + 1) * P)
                qsl = slice(qs * 512, (qs + 1) * 512)
                pq = pqp.tile([P, 512], F32, name=f"pq{c2}_{qs}", tag="pq")
                nc.tensor.matmul(
                    pq, W2T[0][:, csl], fcr[0][:, qsl], start=True, stop=False
                )
                nc.tensor.matmul(
                    pq, W2T[1][:, csl], fcr[1][:, qsl], start=False, stop=True
                )
                if qs % 2 == 0:
                    nc.vector.tensor_scalar(
                        Qs[c2][:, qsl], pq, iB[c2], b2s[c2],
                        op0=OP.mult, op1=OP.add,
                    )
                else:
                    nc.scalar.activation(
                        Qs[c2][:, qsl], pq, AF.Identity,
                        bias=b2s[c2], scale=iB[c2],
                    )
        pqp_cm.__exit__(None, None, None)
        pvp_cm.__exit__(None, None, None)
        pfc_cm.__exit__(None, None, None)

        # ---------------- attention ------------------------------------
        with tc.tile_pool(name="pfcs", bufs=1) as pfcs, tc.tile_pool(
            name="sts", bufs=5
        ) as sts, tc.tile_pool(name="epi", bufs=3) as epi, tc.tile_pool(
            name="psl", bufs=3, space="PSUM"
        ) as psl, tc.tile_pool(name="pmv", bufs=1, space="PSUM") as pmv, tc.tile_pool(
            name="pv2", bufs=1, space="PSUM"
        ) as pv2:
            fcsh_t = pfcs.tile([P, TH // P, D], F32, name="fcsh", tag="fcsh")
            m_bc = pfcs.tile([P, D], F32, name="mbc", tag="mbc")
            i_bc = pfcs.tile([P, D], F32, name="ibc", tag="ibc")
            acc_s_cs = [pfcs.tile([P, NCK], F32, name=f"ascs{c}", tag=f"ascs{c}") for c in range(CH)]
            acc_q_cs = [pfcs.tile([P, NCK], F32, name=f"aqcs{c}", tag=f"aqcs{c}") for c in range(CH)]
            csqA = pfcs.tile([P, CK], F32, name="csqA", tag="csqA")  # ACT dead
            csqB = pfcs.tile([P, CK], F32, name="csqB", tag="csqB")  # DVE dead
            mC = [pfcs.tile([P, 1], F32, name=f"mc{c}", tag=f"mc{c}") for c in range(CH)]
            vC = [pfcs.tile([P, 1], F32, name=f"vc{c}", tag=f"vc{c}") for c in range(CH)]
            iC = [pfcs.tile([P, 1], F32, name=f"ic{c}", tag=f"ic{c}") for c in range(CH)]

            # fcs chunk DMAs + contiguous fcsh load queue behind fs on sync
            cs_ck = []
            for k in range(NCK):
                for c in range(CH):
                    t = pfcs.tile([P, CK], F32, name=f"csck{k}{c}", tag="csck", bufs=3)
                    nc.sync.dma_start(
                        out=t, in_=fcsT[c * P : (c + 1) * P, k * CK : (k + 1) * CK]
                    )
                    cs_ck.append(t)
            nc.sync.dma_start(
                out=fcsh_t, in_=fcsh[:, :].rearrange("p (n d) -> p n d", d=D)
            )

            def emit_cs_stats(i):
                k, c = divmod(i, CH)
                nc.scalar.activation(
                    csqA, cs_ck[i], AF.Copy, accum_out=acc_s_cs[c][:, k : k + 1]
                )
                nc.vector.scalar_tensor_tensor(
                    csqB, cs_ck[i], 0.0, cs_ck[i],
                    op0=OP.add, op1=OP.mult, accum_out=acc_q_cs[c][:, k : k + 1],
                )

            def emit_cs_reduce():
                for c in range(CH):
                    nc.vector.reduce_sum(mC[c], acc_s_cs[c], axis=mybir.AxisListType.X)
                    nc.vector.tensor_scalar_mul(mC[c], mC[c], 1.0 / T)
                    nc.vector.reduce_sum(vC[c], acc_q_cs[c], axis=mybir.AxisListType.X)
                    nc.vector.tensor_scalar_mul(vC[c], vC[c], 1.0 / T)
                    nc.vector.tensor_mul(iC[c], mC[c], mC[c])
                    nc.vector.tensor_sub(vC[c], vC[c], iC[c])
                    nc.scalar.activation(vC[c], vC[c], AF.Ln, bias=eps_t)
                    nc.scalar.activation(iC[c], vC[c], AF.Exp, scale=-0.5)
                    nc.gpsimd.dma_start(out=scm[0, c * P : (c + 1) * P], in_=mC[c])
                    nc.gpsimd.dma_start(out=scm[1, c * P : (c + 1) * P], in_=iC[c])

            def emit_cs_bcast():
                nc.gpsimd.dma_start(out=m_bc, in_=_bcast_row(scm, 0, D))
                nc.gpsimd.dma_start(out=i_bc, in_=_bcast_row(scm, D, D))

            tasks = [(q, jp) for q in range(NQ) for jp in range(NJP)]
            st_map = {}
            mv_map = {}

            def emit_logits(t):
                q, jp = t
                qsl = slice(q * 256, (q + 1) * 256)
                pl = psl.tile([P, 512], F32, name=f"pl{q}_{jp}", tag="pl")
                for h in range(2):
                    j = 2 * jp + h
                    osl = slice(h * 256, (h + 1) * 256)
                    jsl = slice(j * P, (j + 1) * P)
                    nc.tensor.matmul(
                        pl[:, osl], fsr[0][:, jsl], Qs[0][:, qsl],
                        start=True, stop=False,
                    )
                    nc.tensor.matmul(
                        pl[:, osl], fsr[1][:, jsl], Qs[1][:, qsl],
                        start=False, stop=True,
                    )
                st = sts.tile([P, 512], F32R, name="st", tag="st")
                nc.scalar.activation(st, pl, AF.Exp, bias=negc0_t)
                st_map[t] = st

            def get_mv(q):
                if q not in mv_map:
                    mv_map[q] = (
                        [pmv.tile([P, D + 2], F32, name=f"mv{q}_{b}", tag=f"mv{b}") for b in range(2)],
                        [pv2.tile([P, D], F32, name=f"v2_{q}_{b}", tag=f"v2{b}") for b in range(2)],
                    )
                return mv_map[q]

            def emit_av(t):
                q, jp = t
                mv, v2 = get_mv(q)
                st = st_map.pop(t)
                for h in range(2):
                    j = 2 * jp + h
                    for b in range(2):
                        lhs = st[:, h * 256 + b * P : h * 256 + (b + 1) * P]
                        nc.tensor.matmul(
                            mv[b], lhs, Vr[:, j, :],
                            start=(j == 0), stop=(j == NB - 1),
                        )
                        nc.tensor.matmul(
                            v2[b], lhs, V2r[:, j, :],
                            start=(j == 0), stop=(j == NB - 1),
                        )

            def emit_epilogue(q, b):
                mv, v2 = mv_map[q]
                qb = q * 2 + b
                # PSUM-freeing copies FIRST (mve on DVE, v2e on ACT): the
                # next chunk's AV matmuls wait only on these two.
                mve = epi.tile([P, D + 2], F32, name="mve", tag="mve")
                nc.vector.tensor_copy(mve, mv[b])
                v2e = epi.tile([P, D], F32, name="v2e", tag="v2e")
                nc.scalar.activation(v2e, v2[b], AF.Copy)
                recip = epi.tile([P, 1], F32, name="recip", tag="recip")
                nc.vector.reciprocal(recip, mve[:, D : D + 1])
                Mt = epi.tile([P, D], F32, name="Mt", tag="Mt")
                nc.vector.tensor_scalar_mul(Mt, mve[:, 0:D], recip)
                Msq = epi.tile([P, D], F32, name="Msq", tag="Msq")
                nc.vector.tensor_mul(Msq, Mt, Mt)
                # Var -> v2e (in place), clamp, S = Exp(0.5*Ln(Var))
                nc.vector.scalar_tensor_tensor(
                    v2e, v2e, recip, Msq, op0=OP.mult, op1=OP.subtract
                )
                nc.vector.tensor_scalar_max(v2e, v2e, EPS_VAR)
                Sx = epi.tile([P, D], F32, name="Sx", tag="Sx")
                nc.scalar.activation(Sx, v2e, AF.Ln)
                nc.scalar.activation(Sx, Sx, AF.Exp, scale=0.5)
                # fold the fcs instance norm in: out = Sa*fcs + (M + bh - Sa*m)
                Sa = epi.tile([P, D], F32, name="Sa", tag="Sa")
                nc.vector.tensor_mul(Sa, Sx, i_bc)
                t2 = epi.tile([P, D], F32, name="t2", tag="t2")
                nc.vector.tensor_mul(t2, Sa, m_bc)
                Mb = epi.tile([P, D], F32, name="Mb", tag="Mb")
                nc.vector.tensor_add(Mb, Mt, bv_bc)
                nc.vector.tensor_sub(Mb, Mb, t2)
                o1 = epi.tile([P, D], F32, name="o1", tag="o1")
                nc.vector.tensor_mul(o1, Sa, fcsh_t[:, qb, :])
                nc.vector.tensor_add(o1, o1, Mb)
                nc.gpsimd.dma_start(out=out_e[qb * P : (qb + 1) * P, :], in_=o1)

            # deferred V work (fs chunk 3 = pairs 12..15) interleaved into
            # the early task stream; pair pr is needed by av(0, jp=pr) so
            # emission at tasks 0..3 runs ~9 tasks ahead. PSUM comes from
            # the pl pool (same [128,512] shape) — pvp is closed by now.
            def emit_v_deferred(pr):
                v_pair_evac(pr, v_pair(pr, pool=psl, tag="pl"))

            # warm-up the logits pipeline
            for t in range(LA):
                emit_logits(tasks[t])
            for ti, t in enumerate(tasks):
                q, jp = t
                if ti + LA < len(tasks):
                    emit_logits(tasks[ti + LA])
                emit_av(t)
                if q == 0:
                    if jp < 4:
                        emit_v_deferred(12 + jp)
                    elif 5 <= jp <= 12:
                        emit_cs_stats(jp - 5)
                    elif jp == 13:
                        emit_cs_reduce()
                    elif jp == 14:
                        emit_cs_bcast()
                if jp == NJP - 1:
                    emit_epilogue(q, 0)
                    emit_epilogue(q, 1)

        persist_cm.__exit__(None, None, None)

    nc.compile()
    return nc


_CACHE = {}


def _get_nc():
    if "nc" not in _CACHE:
        _CACHE["nc"] = build_nc()
    return _CACHE["nc"]


def kernel(**inputs):
    fc = np.ascontiguousarray(np.asarray(inputs["fc"], dtype=np.float32))
    fs = np.ascontiguousarray(np.asarray(inputs["fs"], dtype=np.float32))
    fcs = np.ascontiguousarray(np.asarray(inputs["fcs"], dtype=np.float32))
    Wf = np.asarray(inputs["Wf"], dtype=np.float32)
    bf = np.asarray(inputs["bf"], dtype=np.float32)
    Wh = np.asarray(inputs["Wh"], dtype=np.float32)
    bh = np.asarray(inputs["bh"], dtype=np.float32)
    Wg = np.asarray(inputs["Wg"], dtype=np.float32)

    wfO = np.ascontiguousarray(Wf)
    wfT = np.ascontiguousarray(Wf.T)
    wgO = np.ascontiguousarray(Wg)
    whT = np.ascontiguousarray(Wh.T)
    bq = np.ascontiguousarray(bf.reshape(D, 1))

    in_maps = []
    for core in range(8):
        s, h = divmod(core, 2)
        fcT_s = fc[s].T  # (D, T)
        if h == 0:
            fcT_perm = np.ascontiguousarray(fcT_s)
        else:
            fcT_perm = np.ascontiguousarray(
                np.concatenate([fcT_s[:, TH:], fcT_s[:, :TH]], axis=1)
            )
        # [TH, D] -> [P, (TH//P)*D]: token block on the middle axis so the
        # device-side load is contiguous per partition
        fcsh_t = np.ascontiguousarray(
            fcs[s, h * TH : (h + 1) * TH, :]
            .reshape(TH // P, P, D)
            .transpose(1, 0, 2)
            .reshape(P, (TH // P) * D)
        )
        in_maps.append(
            {
                "fcT": fcT_perm,
                "fsT": np.ascontiguousarray(fs[s].T),
                "fcsT": np.ascontiguousarray(fcs[s].T),
                "fcsh": fcsh_t,
                "wfO": wfO,
                "wfT": wfT,
                "wgO": wgO,
                "whT": whT,
                "bq": bq,
                "bv": bh,
            }
        )

    nc = _get_nc()
    res = run_bass_kernel_spmd(
        nc, in_maps, core_ids=list(range(8)), trace=TRACE, **TRACE_KW
    )
    if TRACE:
        _CACHE["last_result"] = res

    out = np.empty((4, T, D), np.float32)
    for core in range(8):
        s, h = divmod(core, 2)
        out[s, h * TH : (h + 1) * TH, :] = res.results[core]["out"]
    return out
